# revision 9
# baseline (speedup 1.0000x reference)
"""Erosion (5x5 sliding-window min, geodesic border pad 1e4) on TRN2.

Layout: partition p holds rows 8p-2 .. 8p+9 of one image as 12
contiguous 1024-col bf16 segments, loaded via overlapping-window SWDGE
cast-DMAs (f32 DRAM -> bf16 SBUF, one fat contiguous-read descriptor
per partition; partitions 0/127 get clamped single-descriptor loads,
pad rows come from a 1e4 const tile, pre-filled once per x buffer
since no load ever touches the pad regions). Cast-DMA measured at
~340 GB/s with ~0.8us triggers, vs ~94us/image of HWDGE
descriptor-generation stall for a 4KB-run layout.

Compute is full-width bf16 on DVE (2 elem/cycle/lane, ~0.53ns/elem per
partition, shifted operands included): vertical min = w2/w4/v cascade
along the segment axis (w4 in-place into w2; in1 reads run 2 segments
ahead of the write pointer - pipeline-safe), horizontal min = a/b
cascade plus one-column edge TTs for the geodesic border (no column
pads). Stores cast bf16 -> f32 in the DMA (SWDGE), contiguous
per-partition descriptors.

Pipelining: the GpSimd (SWDGE) queue is IN-ORDER, so a store trigger
that waits on compute would block later load triggers. Issue order is
software-pipelined: loads for stream position k+2 are issued after the
stores of position k; 3 x-buffers / 2 out-buffers. Each image computes
in two 4-output-seg groups (group 1 needs only input segs 0..7); the
first image's segs 0..7 load is further split for a shorter pipeline
fill, the last image's second half stores in two 2-seg pieces for a
shorter drain. bf16 rounding keeps rel err ~2e-3 (tolerance 2e-2).
DVE is the critical path at ~28us/image.
"""

import numpy as np

import concourse.bacc as bacc
import concourse.mybir as mybir
import concourse.tile as tile
from concourse.bass import AP
from concourse.bass_utils import run_bass_kernel_spmd

B, H, W = 32, 1024, 1024
N_CORES = 8
PER_CORE = B // N_CORES     # 4 images per core
PX = 2
PAD_VAL = 1e4
F32 = mybir.dt.float32
BF16 = mybir.dt.bfloat16
MIN = mybir.AluOpType.min

KR = 8                      # output rows per partition (128*8 = 1024)
SEGS = KR + 2 * PX          # 12 segments per partition
GA = 8                      # input segs needed by compute group 1

_CACHE = {}


def build_nc(repeat: int = 1):
    nc = bacc.Bacc("TRN2", debug=False, num_devices=N_CORES)
    x = nc.dram_tensor("mask", [PER_CORE, H, W], F32, kind="ExternalInput").ap()
    y = nc.dram_tensor("out", [PER_CORE, H, W], F32, kind="ExternalOutput").ap()

    N = repeat * PER_CORE   # flat image stream

    with tile.TileContext(nc) as tc:
        with (
            tc.tile_pool(name="const", bufs=1) as cpool,
            tc.tile_pool(name="xp", bufs=1) as xpool,
            tc.tile_pool(name="wp", bufs=1) as wpool,
            tc.tile_pool(name="op", bufs=1) as opool,
        ):
            # 1e4 source for row-pad fills (memset can't start at
            # partition 127; DMA is exempt from start-partition rules)
            cpad = cpool.tile([128, PX * W], BF16)
            nc.vector.memset(cpad[:, :], PAD_VAL)

            # manual buffers; pad regions (p0 segs 0,1 / p127 segs
            # 10,11) are written ONLY here, so fill them once
            xbufs, obufs = [], []
            for i in range(3):
                xb = xpool.tile([128, SEGS * W], BF16, tag=f"x{i}", name=f"xb{i}")
                nc.sync.dma_start(out=xb[0:1, 0 : PX * W], in_=cpad[0:1, :])
                nc.sync.dma_start(
                    out=xb[127:128, (SEGS - PX) * W : SEGS * W], in_=cpad[0:1, :]
                )
                xbufs.append(xb)
            for i in range(2):
                obufs.append(
                    opool.tile([128, KR * W], BF16, tag=f"o{i}", name=f"ob{i}")
                )
            w2 = wpool.tile([128, (SEGS - 2) * W], BF16, tag="w2")
            w2_3 = w2[:, :].rearrange("p (s c) -> p s c", s=SEGS - 2)
            v = wpool.tile([128, KR * W], BF16, tag="v")
            v3 = v[:, :].rearrange("p (s c) -> p s c", s=KR)
            aa = wpool.tile([128, KR * W], BF16, tag="a")
            a3 = aa[:, :].rearrange("p (s c) -> p s c", s=KR)
            bb = wpool.tile([128, KR * W], BF16, tag="b")
            b3 = bb[:, :].rearrange("p (s c) -> p s c", s=KR)

            def issue_loads(k):
                """SWDGE cast loads (f32->bf16) for stream position k."""
                img = k % PER_CORE
                xb = xbufs[k % 3]
                base = img * H * W
                # p0: segs 2..11 <- rows 0..9 (one descriptor)
                nc.gpsimd.dma_start(
                    out=xb[0:1, PX * W : SEGS * W],
                    in_=AP(
                        x.tensor, base, [[(SEGS - PX) * W, 1], [1, (SEGS - PX) * W]]
                    ),
                )
                # p127: segs 0..9 <- rows 1014..1023
                nc.gpsimd.dma_start(
                    out=xb[127:128, 0 : (SEGS - PX) * W],
                    in_=AP(
                        x.tensor,
                        base + (H - (SEGS - PX)) * W,
                        [[(SEGS - PX) * W, 1], [1, (SEGS - PX) * W]],
                    ),
                )
                # main overlap loads, partitions 1..126: segs 0..7 then
                # 8..11 (rows 8p-2+s); first image splits segs 0..7 in
                # two (5+3: the bigger first chunk's w2 covers the wait
                # for the second) for a shorter pipeline fill
                halves = [(0, 5), (5, 3)] if k == 0 else [(0, GA)]
                for lo, n in halves:
                    nc.gpsimd.dma_start(
                        out=xb[1:127, lo * W : (lo + n) * W],
                        in_=AP(
                            x.tensor,
                            base + (KR - PX + lo) * W,
                            [[KR * W, 126], [1, n * W]],
                        ),
                    )
                nc.gpsimd.dma_start(
                    out=xb[1:127, GA * W : SEGS * W],
                    in_=AP(
                        x.tensor,
                        base + (KR - PX + GA) * W,
                        [[KR * W, 126], [1, (SEGS - GA) * W]],
                    ),
                )

            def horizontal(o3, sl):
                """a/b cascade + geodesic edge columns for out segs sl."""
                nc.vector.tensor_tensor(
                    out=a3[:, sl, 0 : W - 1],
                    in0=v3[:, sl, 0 : W - 1],
                    in1=v3[:, sl, 1:W],
                    op=MIN,
                )
                nc.vector.tensor_tensor(
                    out=b3[:, sl, 0 : W - 3],
                    in0=a3[:, sl, 0 : W - 3],
                    in1=a3[:, sl, 2 : W - 1],
                    op=MIN,
                )
                nc.vector.tensor_tensor(
                    out=o3[:, sl, PX : W - PX],
                    in0=b3[:, sl, 0 : W - 2 * PX],
                    in1=v3[:, sl, 2 * PX : W],
                    op=MIN,
                )
                nc.vector.tensor_tensor(
                    out=o3[:, sl, 0:1], in0=a3[:, sl, 0:1], in1=a3[:, sl, 1:2], op=MIN
                )
                nc.vector.tensor_tensor(
                    out=o3[:, sl, 1:2], in0=b3[:, sl, 0:1], in1=v3[:, sl, 0:1], op=MIN
                )
                nc.vector.tensor_tensor(
                    out=o3[:, sl, W - 2 : W - 1],
                    in0=b3[:, sl, W - 4 : W - 3],
                    in1=v3[:, sl, W - 2 : W - 1],
                    op=MIN,
                )
                nc.vector.tensor_tensor(
                    out=o3[:, sl, W - 1 : W],
                    in0=a3[:, sl, W - 3 : W - 2],
                    in1=a3[:, sl, W - 2 : W - 1],
                    op=MIN,
                )

            def store(k, s0, nseg):
                """SWDGE cast store (bf16->f32): out segs s0..s0+nseg-1,
                partition p -> rows 8p+s0 .. (contiguous descriptors)."""
                img = k % PER_CORE
                ob = obufs[k % 2]
                nc.gpsimd.dma_start(
                    out=AP(
                        y.tensor,
                        img * H * W + s0 * W,
                        [[KR * W, 128], [1, nseg * W]],
                    ),
                    in_=ob[:, s0 * W : (s0 + nseg) * W],
                )

            # prologue: loads for the first two stream positions
            issue_loads(0)
            if N > 1:
                issue_loads(1)

            for k in range(N):
                xb = xbufs[k % 3]
                ob = obufs[k % 2]
                x3 = xb[:, :].rearrange("p (s c) -> p s c", s=SEGS)
                o3 = ob[:, :].rearrange("p (s c) -> p s c", s=KR)

                # ---- group 1: out segs 0..3 (needs x segs 0..7) ----
                # w2[s]=min(x[s],x[s+1]); split w2 on the first image to
                # start after the first quarter-load
                if k == 0:
                    nc.vector.tensor_tensor(
                        out=w2_3[:, 0:4, :], in0=x3[:, 0:4, :], in1=x3[:, 1:5, :],
                        op=MIN,
                    )
                    nc.vector.tensor_tensor(
                        out=w2_3[:, 4:7, :], in0=x3[:, 4:7, :], in1=x3[:, 5:8, :],
                        op=MIN,
                    )
                else:
                    nc.vector.tensor_tensor(
                        out=w2_3[:, 0:7, :], in0=x3[:, 0:7, :], in1=x3[:, 1:8, :],
                        op=MIN,
                    )
                # w4[0..4] in place (one extra for group 2's v[4])
                nc.vector.tensor_tensor(
                    out=w2_3[:, 0:5, :], in0=w2_3[:, 0:5, :], in1=w2_3[:, 2:7, :],
                    op=MIN,
                )
                nc.vector.tensor_tensor(
                    out=v3[:, 0:4, :], in0=w2_3[:, 0:4, :], in1=x3[:, 4:8, :],
                    op=MIN,
                )
                horizontal(o3, slice(0, 4))
                store(k, 0, 4)

                # ---- group 2: out segs 4..7 (adds x segs 8..11) ----
                # w2[7..9], w4[5..7], v[4..7]; w2[10]/w4[8] are never
                # consumed (v[7] = min(w4[7], x[11]) covers segs 7..11)
                nc.vector.tensor_tensor(
                    out=w2_3[:, 7:10, :], in0=x3[:, 7:10, :], in1=x3[:, 8:11, :],
                    op=MIN,
                )
                nc.vector.tensor_tensor(
                    out=w2_3[:, 5:8, :], in0=w2_3[:, 5:8, :], in1=w2_3[:, 7:10, :],
                    op=MIN,
                )
                nc.vector.tensor_tensor(
                    out=v3[:, 4:8, :], in0=w2_3[:, 4:8, :], in1=x3[:, 8:12, :],
                    op=MIN,
                )
                if k == N - 1:
                    # shorter drain: two 2-seg pieces
                    horizontal(o3, slice(4, 6))
                    store(k, 4, 2)
                    horizontal(o3, slice(6, 8))
                    store(k, 6, 2)
                else:
                    horizontal(o3, slice(4, 8))
                    store(k, 4, 4)

                # loads for stream position k+2 AFTER this image's
                # stores (keeps the in-order SWDGE queue flowing)
                if k + 2 < N:
                    issue_loads(k + 2)

    nc.compile()
    return nc


def run(mask: np.ndarray, trace: bool = False, tmpdir: str | None = None):
    assert mask.shape == (B, 1, H, W), mask.shape
    in_dtype = mask.dtype
    mask4 = np.ascontiguousarray(
        mask.reshape(B, H, W).astype(np.float32, copy=False)
    )
    if "nc" not in _CACHE:
        _CACHE["nc"] = build_nc(1)
    nc = _CACHE["nc"]
    in_maps = [
        {"mask": mask4[i * PER_CORE : (i + 1) * PER_CORE]} for i in range(N_CORES)
    ]
    res = run_bass_kernel_spmd(
        nc, in_maps, list(range(N_CORES)), trace=trace, tmpdir=tmpdir
    )
    out = np.concatenate([res.results[i]["out"] for i in range(N_CORES)], axis=0)
    return out.reshape(B, 1, H, W).astype(in_dtype, copy=False), res


def kernel(mask: np.ndarray) -> np.ndarray:
    return run(mask)[0]
